# revision 27
# baseline (speedup 1.0000x reference)
"""Trainium2 Bass kernel for nn_CausalPredictor.

Reference math (per image y = x[b], all f32):
    zd   = dic @ Wz_w.T + Wz_b                          [K, C]
    att  = softmax((y @ Wy_w.T + Wy_b) @ zd.T * s, k)   [L, K]
    z    = (att * prior) @ dic                          [L, D]
    ly   = y @ cs_w[:, :D].T                            [L, C]
    lz   = z @ cs_w[:, D:].T + cs_b                     [L, C]
    out[i*L+j, c] = ly[i, c] + lz[j, c]                 [L*L, C]

The weight-only algebra is tiny (O(D*K*C)) and is folded on the HOST:
    zdts  = (zd.T + Wz_b) * s                 [C, K]
    M     = Wy_w.T @ zdts                     [D, K]
    ebias = Wy_b @ zdts                       [K]     (exp bias column)
    gb    = [diag(prior) @ dic @ csz.T | 1]   [K, C+1]
    csyT  = cs_w[:, :D].T                     [D, C]
and y is passed pre-transposed (yT, contraction dim on partitions), so the
device graph is just:
    ep   = exp(M.T @ yT + ebias)              [K, L]
    nd   = ep_slice.T @ gb                    [128j, C+1]  (num | denom)
    lz   = nd[:, :C] / nd[:, C:] + cs_b       [128j, C]
    lyT  = csyT.T @ yT[:, :512]               [C, 512]
    out block = lhsT.T @ rhs                  (K=99 bf16 matmul per block)

The outer sum runs on the PE in bf16 at 1 cycle/row (fp32 matmul is 4 via
LOW_HIGH): ly and lz are each split into hi+mid+lo bf16 mantissa parts, the
rhs holds three copies of a tiled identity (rows 32s..32s+20) plus the three
lz_flat splits (rows 96..98), and the lhsT holds the three lyT splits plus
ones rows (96..98).  Every product is value * {0,1} (exact in bf16),
accumulated in fp32 PSUM, so the result is fp32-exact to ~2^-24.

Sharding: 8 cores = 4 images x 2 halves of the i dim, no collectives.  The
host hands each core yT with its OWN i-half's columns first, so those
columns feed both the ly path and the first j-half's attention path; the
host un-permutes the j-halves when assembling.  The second half's compute
hides under the first half's output DMA (the hard floor: ~44 MB of output
writes per core).
"""

import sys

for _p in ("/opt/trn_rl_repo", "/root/.axon_site/_ro/trn_rl_repo"):
    if _p not in sys.path:
        sys.path.append(_p)

import numpy as np

import concourse.bass as bass
from concourse import bacc
import concourse.mybir as mybir
import concourse.tile as tile
from contextlib import ExitStack

B, L, D, K, C = 4, 1024, 1024, 20, 21
SCALE = 1.0 / float(np.sqrt(np.float32(C)))
F32 = mybir.dt.float32
BF16 = mybir.dt.bfloat16
HALF_L = L // 2          # 512 rows of i per core
N_IC = HALF_L // 128     # 4 i-chunks of 128 per core
N_DC = D // 128          # 8 chunks along the contraction dim
JC = 512                 # j columns covered by one rhs tile (one j-half)
RHS_W = JC * C           # 10752 free elements per rhs tile
Q_N = RHS_W // 512       # 21 matmuls of N=512 per (half, ic)
OUT_Q = 7                # q's per staged output tile
OUT_W = OUT_Q * 512      # 3584 f32 per partition per staged tile


def _build_program():
    nc = bacc.Bacc(
        "TRN2",
        target_bir_lowering=False,
        debug=False,
        enable_asserts=False,
        num_devices=8,
    )
    yt_d = nc.dram_tensor("yT", [D, L], F32, kind="ExternalInput").ap()
    m_d = nc.dram_tensor("M", [128, N_DC, K], F32, kind="ExternalInput").ap()
    csy_d = nc.dram_tensor("csyT", [128, N_DC, C], F32, kind="ExternalInput").ap()
    gb_d = nc.dram_tensor("gb", [K, C + 1], F32, kind="ExternalInput").ap()
    eb_d = nc.dram_tensor("ebias", [K], F32, kind="ExternalInput").ap()
    csb_d = nc.dram_tensor("cs_b", [C], F32, kind="ExternalInput").ap()
    icorner_d = nc.dram_tensor("icorner", [32, RHS_W], BF16, kind="ExternalInput").ap()
    out = nc.dram_tensor("out_loc", [HALF_L, L * C], F32, kind="ExternalOutput").ap()

    with tile.TileContext(nc) as tc:
        _emit(tc, out, yt_d, m_d, csy_d, gb_d, eb_d, csb_d, icorner_d)
    nc.compile()
    return nc


def _bcast_ap(ap, parts):
    """Partition-broadcast a 1-D DRAM AP across `parts` partitions (DMA only)."""
    return bass.AP(tensor=ap.tensor, offset=ap.offset, ap=[[0, parts]] + list(ap.ap))


def _emit(tc, out, yt_d, m_d, csy_d, gb_d, eb_d, csb_d, icorner_d):
    nc = tc.nc
    ctx = ExitStack()
    with ctx:
        consts = ctx.enter_context(tc.tile_pool(name="consts", bufs=1))
        outpool = ctx.enter_context(tc.tile_pool(name="outpool", bufs=4))
        small = ctx.enter_context(tc.tile_pool(name="small", bufs=2))
        # PSUM: sm 2 banks + out 6 banks = 8, no pool releases.
        sm_ps = ctx.enter_context(tc.tile_pool(name="sm_ps", bufs=2, space="PSUM"))
        out_ps = ctx.enter_context(tc.tile_pool(name="out_ps", bufs=6, space="PSUM"))

        def sm_tile(p, f):
            return sm_ps.tile([p, f], F32, name="sm", tag="sm")

        # ---- constant loads (ACT ring; sync ring is reserved for output) ----
        m_sb = consts.tile([128, N_DC, K], F32, name="m_sb")
        nc.scalar.dma_start(out=m_sb, in_=m_d)
        csy_sb = consts.tile([128, N_DC, C], F32, name="csy_sb")
        nc.scalar.dma_start(out=csy_sb, in_=csy_d)
        gb = consts.tile([K, C + 1], F32, name="gb")
        nc.scalar.dma_start(out=gb, in_=gb_d)
        ebias = consts.tile([K, 1], F32, name="ebias")
        nc.scalar.dma_start(out=ebias, in_=eb_d.unsqueeze(1))
        csb_rep = consts.tile([128, C], F32, name="csb_rep")
        nc.scalar.dma_start(out=csb_rep, in_=_bcast_ap(csb_d, 128))

        # yT tiles: first-half columns on the ACT ring (critical path), second
        # half on the gpsimd ring after the rhs prep.
        yT = [consts.tile([128, L], F32, name=f"yT{dc}") for dc in range(N_DC)]
        for dc in range(N_DC):
            nc.scalar.dma_start(
                out=yT[dc][:, 0:JC],
                in_=yt_d[dc * 128 : (dc + 1) * 128, 0:JC],
            )

        # rhs tiles: rows 32s..32s+20 = tiled I_C per bf16 split (host
        # constant), rows 96..98 = this half's lz_flat splits.
        rhs = [consts.tile([99, RHS_W], BF16, name=f"rhs{h}") for h in range(2)]
        nc.sync.dma_start(out=rhs[0][0:32, :], in_=icorner_d)
        nc.sync.dma_start(out=rhs[0][32:64, :], in_=rhs[0][0:32, :])
        nc.sync.dma_start(out=rhs[0][64:96, :], in_=rhs[0][0:32, :])
        for dc in range(N_DC):
            nc.gpsimd.dma_start(
                out=yT[dc][:, JC:L],
                in_=yt_d[dc * 128 : (dc + 1) * 128, JC:L],
            )
        nc.gpsimd.dma_start(out=rhs[1][0:96, :], in_=rhs[0][0:96, :])

        # PE warmup: ~25 dependency-free bf16 matmuls so the HAM releases the
        # clock gate before the real (latency-critical) matmuls arrive.
        warm = consts.tile([128, 640], BF16, name="warm")
        nc.vector.memset(warm, 0.0)
        for _ in range(25):
            pw = out_ps.tile([128, 512], F32, name="pw", tag="po")
            nc.tensor.matmul(pw, warm[:, 0:128], warm[:, 128:640])

        # ly lhsT skeleton (rows filled per chunk below)
        ly_lhsT = consts.tile([99, HALF_L], BF16, name="ly_lhsT")
        nc.vector.memset(ly_lhsT, 0.0)
        nc.vector.memset(ly_lhsT[96:99, :], 1.0)

        ep = consts.tile([K, L], F32, name="ep")
        lz_sb = consts.tile([128, N_DC, C], F32, name="lz_sb")
        lz_hi = consts.tile([128, N_DC, C], BF16, name="lz_hi")
        lz_mid = consts.tile([128, N_DC, C], BF16, name="lz_mid")
        lz_lo = consts.tile([128, N_DC, C], BF16, name="lz_lo")
        lz_rem1 = consts.tile([128, N_DC, C], F32, name="lz_rem1")
        lz_rem2 = consts.tile([128, N_DC, C], F32, name="lz_rem2")
        lz_splits = [lz_hi, lz_mid, lz_lo]

        def half_ep(h):
            """ep[:, half] = exp(M.T @ yT_half + ebias)."""
            jsl = slice(h * JC, (h + 1) * JC)
            ps_u = sm_tile(K, JC)
            for dc in range(N_DC):
                nc.tensor.matmul(ps_u, m_sb[:, dc, :], yT[dc][:, jsl],
                                 start=(dc == 0), stop=(dc == N_DC - 1))
            nc.scalar.activation(ep[:, jsl], ps_u,
                                 mybir.ActivationFunctionType.Exp,
                                 bias=ebias, scale=1.0)

        def chunk_lz(lc):
            """lz chunk lc + bf16 splits + flatten into rhs rows 96..98."""
            h, lc4 = lc // 4, lc % 4
            csl = slice(lc * 128, (lc + 1) * 128)
            ps_nd = sm_tile(128, C + 1)
            nc.tensor.matmul(ps_nd, ep[:, csl], gb)
            recip = small.tile([128, 1], F32, name="recip", tag="recip")
            nc.vector.reciprocal(recip, ps_nd[:, C : C + 1])
            nc.vector.scalar_tensor_tensor(
                lz_sb[:, lc, :], ps_nd[:, 0:C], recip, csb_rep,
                op0=mybir.AluOpType.mult, op1=mybir.AluOpType.add)
            nc.vector.tensor_copy(lz_hi[:, lc, :], lz_sb[:, lc, :])
            nc.vector.tensor_sub(lz_rem1[:, lc, :], lz_sb[:, lc, :],
                                 lz_hi[:, lc, :])
            nc.vector.tensor_copy(lz_mid[:, lc, :], lz_rem1[:, lc, :])
            nc.vector.tensor_sub(lz_rem2[:, lc, :], lz_rem1[:, lc, :],
                                 lz_mid[:, lc, :])
            nc.vector.tensor_copy(lz_lo[:, lc, :], lz_rem2[:, lc, :])
            for s in range(3):
                nc.sync.dma_start(
                    out=rhs[h][96 + s : 97 + s, lc4 * 128 * C : (lc4 + 1) * 128 * C],
                    in_=lz_splits[s][:, lc, :],
                )

        def chunk_ly(ic):
            """lyT cols ic*128.. -> bf16 splits into ly_lhsT."""
            csl = slice(ic * 128, (ic + 1) * 128)
            ps_lyc = sm_tile(C, 128)
            for dc in range(N_DC):
                nc.tensor.matmul(ps_lyc, csy_sb[:, dc, :], yT[dc][:, csl],
                                 start=(dc == 0), stop=(dc == N_DC - 1))
            hi_b = small.tile([C, 128], BF16, name="hi_b", tag="hi_b")
            mid_b = small.tile([C, 128], BF16, name="mid_b", tag="mid_b")
            lo_b = small.tile([C, 128], BF16, name="lo_b", tag="lo_b")
            rem1 = small.tile([C, 128], F32, name="rem1", tag="rem1")
            rem2 = small.tile([C, 128], F32, name="rem2", tag="rem2")
            nc.scalar.copy(hi_b, ps_lyc)
            nc.vector.tensor_sub(rem1, ps_lyc, hi_b)
            nc.scalar.copy(mid_b, rem1)
            nc.vector.tensor_sub(rem2, rem1, mid_b)
            nc.scalar.copy(lo_b, rem2)
            nc.sync.dma_start(out=ly_lhsT[0:C, csl], in_=hi_b)
            nc.sync.dma_start(out=ly_lhsT[32 : 32 + C, csl], in_=mid_b)
            nc.sync.dma_start(out=ly_lhsT[64 : 64 + C, csl], in_=lo_b)

        def outer_sum(h, ic):
            lhs = ly_lhsT[:, ic * 128 : (ic + 1) * 128]
            for qg in range(Q_N // OUT_Q):
                ob = outpool.tile([128, OUT_W], F32, name="ob", tag="ob")
                for qq in range(OUT_Q):
                    q = qg * OUT_Q + qq
                    po = out_ps.tile([128, 512], F32, name="po", tag="po")
                    nc.tensor.matmul(po, lhs, rhs[h][:, q * 512 : (q + 1) * 512])
                    dst = ob[:, qq * 512 : (qq + 1) * 512]
                    if q % 2 == 0:
                        nc.vector.tensor_copy(dst, po)
                    else:
                        nc.scalar.copy(dst, po)
                nc.sync.dma_start(
                    out=out[ic * 128 : (ic + 1) * 128,
                            h * RHS_W + qg * OUT_W : h * RHS_W + (qg + 1) * OUT_W],
                    in_=ob,
                )

        # h=0 attention + ly, then phase-2 h=0 interleaved with the h=1
        # pipeline (so h=1's small ops don't queue behind a full half's
        # PSUM->SBUF copies on the in-order engine queues), then phase-2 h=1.
        half_ep(0)
        for lc in range(4):
            chunk_lz(lc)
            chunk_ly(lc)
        outer_sum(0, 0)
        half_ep(1)
        outer_sum(0, 1)
        for lc in (4, 5):
            chunk_lz(lc)
        outer_sum(0, 2)
        for lc in (6, 7):
            chunk_lz(lc)
        outer_sum(0, 3)
        for ic in range(N_IC):
            outer_sum(1, ic)


_NC_CACHE = None


def _get_nc():
    global _NC_CACHE
    if _NC_CACHE is None:
        _NC_CACHE = _build_program()
    return _NC_CACHE


def _host_weights(inputs):
    """Fold the weight-only algebra on the host (float64 for headroom)."""
    dic = np.asarray(inputs["dic"], np.float64)
    prior = np.asarray(inputs["prior"], np.float64)
    wy_w = np.asarray(inputs["Wy_w"], np.float64)
    wy_b = np.asarray(inputs["Wy_b"], np.float64)
    wz_w = np.asarray(inputs["Wz_w"], np.float64)
    wz_b = np.asarray(inputs["Wz_b"], np.float64)
    cs_w = np.asarray(inputs["cs_w"], np.float64)
    cs_b = np.asarray(inputs["cs_b"], np.float32)

    zdts = (wz_w @ dic.T + wz_b[:, None]) * float(SCALE)   # [C, K]
    m = (wy_w.T @ zdts).astype(np.float32)                 # [D, K]
    m = np.ascontiguousarray(m.reshape(N_DC, 128, K).transpose(1, 0, 2))
    ebias = (wy_b @ zdts).astype(np.float32)               # [K]
    g = (prior[:, None] * dic) @ cs_w[:, D:].T             # [K, C]
    gb = np.concatenate([g, np.ones((K, 1))], axis=1).astype(np.float32)
    csyT = cs_w[:, :D].T.astype(np.float32)                # [D, C]
    csyT = np.ascontiguousarray(csyT.reshape(N_DC, 128, C).transpose(1, 0, 2))
    import ml_dtypes
    icorner = np.zeros((32, RHS_W), ml_dtypes.bfloat16)
    for c in range(C):
        icorner[c, c::C] = 1.0
    return {
        "icorner": icorner,
        "M": m,
        "csyT": csyT,
        "gb": np.ascontiguousarray(gb),
        "ebias": np.ascontiguousarray(ebias),
        "cs_b": np.ascontiguousarray(cs_b),
    }


def make_in_maps(inputs):
    x = np.asarray(inputs["x"], dtype=np.float32)
    w = _host_weights(inputs)
    xT = [np.ascontiguousarray(x[b].T) for b in range(B)]  # [D, L] each
    in_maps = []
    for core in range(8):
        b, ihalf = core % B, core // B
        if ihalf == 0:
            yt = xT[b]
        else:
            yt = np.ascontiguousarray(
                np.concatenate([xT[b][:, HALF_L:], xT[b][:, :HALF_L]], axis=1)
            )
        in_maps.append({"yT": yt, **w})
    return in_maps


def assemble(results):
    out = np.empty((B, L, L, C), dtype=np.float32)
    for core in range(8):
        b, ihalf = core % B, core // B
        # device output: [512 i_local, 2 processed-half, 512 j_local, C];
        # processed half 0 covers real j-half `ihalf`, half 1 the other.
        r = results[core]["out_loc"].reshape(HALF_L, 2, JC, C)
        dst = out[b, ihalf * HALF_L : (ihalf + 1) * HALF_L]
        dst[:, ihalf * JC : (ihalf + 1) * JC] = r[:, 0]
        dst[:, (1 - ihalf) * JC : (2 - ihalf) * JC] = r[:, 1]
    return out.reshape(B, L * L, C)


def _install_trace_support():
    """The agent image's antenv lacks axon_hooks, so boot() skipped NTFF hook
    install. Recreate the module and register the ctypes-based hook; also stub
    the S3 artifact upload (no creds in this container)."""
    import types

    if sys.modules.get("antenv.axon_hooks") is None:
        mod = types.ModuleType("antenv.axon_hooks")
        _hook = [None]
        mod.set_axon_ntff_profile_hook = lambda h: _hook.__setitem__(0, h)
        mod.get_axon_ntff_profile_hook = lambda: _hook[0]
        sys.modules["antenv.axon_hooks"] = mod
        import antenv

        antenv.axon_hooks = mod
    import antenv.axon_hooks as ah

    if ah.get_axon_ntff_profile_hook() is None:
        from trn_agent_boot.trn_boot import _ntff_profile_via_ctypes

        ah.set_axon_ntff_profile_hook(
            _ntff_profile_via_ctypes("/opt/axon/libaxon_pjrt.so")
        )
    import concourse.bass_utils as bu

    bu.upload_artifacts = lambda tmpdir: tmpdir


def run(inputs, trace=False, **kw):
    from concourse.bass_utils import run_bass_kernel_spmd

    if trace:
        _install_trace_support()
    nc = _get_nc()
    res = run_bass_kernel_spmd(
        nc, make_in_maps(inputs), core_ids=list(range(8)), trace=trace, **kw
    )
    return assemble(res.results), res


def kernel(**inputs) -> np.ndarray:
    out, _ = run(inputs, trace=False)
    return out


# revision 28
# speedup vs baseline: 1.0115x; 1.0115x over previous
"""Trainium2 Bass kernel for nn_CausalPredictor.

Reference math (per image y = x[b], all f32):
    zd   = dic @ Wz_w.T + Wz_b                          [K, C]
    att  = softmax((y @ Wy_w.T + Wy_b) @ zd.T * s, k)   [L, K]
    z    = (att * prior) @ dic                          [L, D]
    ly   = y @ cs_w[:, :D].T                            [L, C]
    lz   = z @ cs_w[:, D:].T + cs_b                     [L, C]
    out[i*L+j, c] = ly[i, c] + lz[j, c]                 [L*L, C]

The weight-only algebra is tiny (O(D*K*C)) and is folded on the HOST:
    zdts  = (zd.T + Wz_b) * s                 [C, K]
    M     = Wy_w.T @ zdts                     [D, K]
    ebias = Wy_b @ zdts                       [K]     (exp bias column)
    gb    = [diag(prior) @ dic @ csz.T | 1]   [K, C+1]
    csyT  = cs_w[:, :D].T                     [D, C]
and y is passed pre-transposed (yT, contraction dim on partitions), so the
device graph is just:
    ep   = exp(M.T @ yT + ebias)              [K, L]
    nd   = ep_slice.T @ gb                    [128j, C+1]  (num | denom)
    lz   = nd[:, :C] / nd[:, C:] + cs_b       [128j, C]
    lyT  = csyT.T @ yT[:, :512]               [C, 512]
    out block = lhsT.T @ rhs                  (K=99 bf16 matmul per block)

The outer sum runs on the PE in bf16 at 1 cycle/row (fp32 matmul is 4 via
LOW_HIGH): ly and lz are each split into hi+mid+lo bf16 mantissa parts, the
rhs holds three copies of a tiled identity (rows 32s..32s+20) plus the three
lz_flat splits (rows 96..98), and the lhsT holds the three lyT splits plus
ones rows (96..98).  Every product is value * {0,1} (exact in bf16),
accumulated in fp32 PSUM, so the result is fp32-exact to ~2^-24.

Sharding: 8 cores = 4 images x 2 halves of the i dim, no collectives.  The
host hands each core yT with its OWN i-half's columns first, so those
columns feed both the ly path and the first j-half's attention path; the
host un-permutes the j-halves when assembling.  The second half's compute
hides under the first half's output DMA (the hard floor: ~44 MB of output
writes per core).
"""

import sys

for _p in ("/opt/trn_rl_repo", "/root/.axon_site/_ro/trn_rl_repo"):
    if _p not in sys.path:
        sys.path.append(_p)

import numpy as np

import concourse.bass as bass
from concourse import bacc
import concourse.mybir as mybir
import concourse.tile as tile
from contextlib import ExitStack

B, L, D, K, C = 4, 1024, 1024, 20, 21
SCALE = 1.0 / float(np.sqrt(np.float32(C)))
F32 = mybir.dt.float32
BF16 = mybir.dt.bfloat16
HALF_L = L // 2          # 512 rows of i per core
N_IC = HALF_L // 128     # 4 i-chunks of 128 per core
N_DC = D // 128          # 8 chunks along the contraction dim
JC = 512                 # j columns covered by one rhs tile (one j-half)
RHS_W = JC * C           # 10752 free elements per rhs tile
Q_N = RHS_W // 512       # 21 matmuls of N=512 per (half, ic)
OUT_Q = 7                # q's per staged output tile
OUT_W = OUT_Q * 512      # 3584 f32 per partition per staged tile


def _build_program():
    nc = bacc.Bacc(
        "TRN2",
        target_bir_lowering=False,
        debug=False,
        enable_asserts=False,
        num_devices=8,
    )
    yt_d = nc.dram_tensor("yT", [D, L], F32, kind="ExternalInput").ap()
    m_d = nc.dram_tensor("M", [128, N_DC, K], F32, kind="ExternalInput").ap()
    csy_d = nc.dram_tensor("csyT", [128, N_DC, C], F32, kind="ExternalInput").ap()
    gb_d = nc.dram_tensor("gb", [K, C + 1], F32, kind="ExternalInput").ap()
    eb_d = nc.dram_tensor("ebias", [K], F32, kind="ExternalInput").ap()
    csb_d = nc.dram_tensor("cs_b", [C], F32, kind="ExternalInput").ap()
    icorner_d = nc.dram_tensor("icorner", [32, RHS_W], BF16, kind="ExternalInput").ap()
    out = nc.dram_tensor("out_loc", [HALF_L, L * C], F32, kind="ExternalOutput").ap()

    with tile.TileContext(nc) as tc:
        _emit(tc, out, yt_d, m_d, csy_d, gb_d, eb_d, csb_d, icorner_d)
    nc.compile()
    return nc


def _bcast_ap(ap, parts):
    """Partition-broadcast a 1-D DRAM AP across `parts` partitions (DMA only)."""
    return bass.AP(tensor=ap.tensor, offset=ap.offset, ap=[[0, parts]] + list(ap.ap))


def _emit(tc, out, yt_d, m_d, csy_d, gb_d, eb_d, csb_d, icorner_d):
    nc = tc.nc
    ctx = ExitStack()
    with ctx:
        consts = ctx.enter_context(tc.tile_pool(name="consts", bufs=1))
        outpool = ctx.enter_context(tc.tile_pool(name="outpool", bufs=4))
        small = ctx.enter_context(tc.tile_pool(name="small", bufs=2))
        # PSUM: sm 2 banks + out 6 banks = 8, no pool releases.
        sm_ps = ctx.enter_context(tc.tile_pool(name="sm_ps", bufs=2, space="PSUM"))
        out_ps = ctx.enter_context(tc.tile_pool(name="out_ps", bufs=6, space="PSUM"))

        def sm_tile(p, f):
            return sm_ps.tile([p, f], F32, name="sm", tag="sm")

        # ---- constant loads (ACT ring; sync ring is reserved for output) ----
        m_sb = consts.tile([128, N_DC, K], F32, name="m_sb")
        nc.scalar.dma_start(out=m_sb, in_=m_d)
        csy_sb = consts.tile([128, N_DC, C], F32, name="csy_sb")
        nc.scalar.dma_start(out=csy_sb, in_=csy_d)
        gb = consts.tile([K, C + 1], F32, name="gb")
        nc.scalar.dma_start(out=gb, in_=gb_d)
        ebias = consts.tile([K, 1], F32, name="ebias")
        nc.scalar.dma_start(out=ebias, in_=eb_d.unsqueeze(1))
        csb_rep = consts.tile([128, C], F32, name="csb_rep")
        nc.scalar.dma_start(out=csb_rep, in_=_bcast_ap(csb_d, 128))

        # yT tiles: first-half columns on the ACT ring (critical path), second
        # half on the gpsimd ring after the rhs prep.
        yT = [consts.tile([128, L], F32, name=f"yT{dc}") for dc in range(N_DC)]
        for dc in range(N_DC):
            nc.scalar.dma_start(
                out=yT[dc][:, 0:JC],
                in_=yt_d[dc * 128 : (dc + 1) * 128, 0:JC],
            )

        # rhs tiles: rows 32s..32s+20 = tiled I_C per bf16 split (host
        # constant), rows 96..98 = this half's lz_flat splits.
        rhs = [consts.tile([99, RHS_W], BF16, name=f"rhs{h}") for h in range(2)]
        nc.sync.dma_start(out=rhs[0][0:32, :], in_=icorner_d)
        nc.sync.dma_start(out=rhs[0][32:64, :], in_=rhs[0][0:32, :])
        nc.sync.dma_start(out=rhs[0][64:96, :], in_=rhs[0][0:32, :])
        nc.gpsimd.dma_start(out=rhs[1][0:96, :], in_=rhs[0][0:96, :])

        # PE warmup: ~25 dependency-free bf16 matmuls so the HAM releases the
        # clock gate before the real (latency-critical) matmuls arrive.
        warm = consts.tile([128, 640], BF16, name="warm")
        nc.vector.memset(warm, 0.0)
        for _ in range(8):
            pw = out_ps.tile([128, 512], F32, name="pw", tag="po")
            nc.tensor.matmul(pw, warm[:, 0:128], warm[:, 128:640])

        # ly lhsT skeleton (rows filled per chunk below)
        ly_lhsT = consts.tile([99, HALF_L], BF16, name="ly_lhsT")
        nc.vector.memset(ly_lhsT, 0.0)
        nc.vector.memset(ly_lhsT[96:99, :], 1.0)

        ep = consts.tile([K, L], F32, name="ep")
        lz_sb = consts.tile([128, N_DC, C], F32, name="lz_sb")
        lz_hi = consts.tile([128, N_DC, C], BF16, name="lz_hi")
        lz_mid = consts.tile([128, N_DC, C], BF16, name="lz_mid")
        lz_lo = consts.tile([128, N_DC, C], BF16, name="lz_lo")
        lz_rem1 = consts.tile([128, N_DC, C], F32, name="lz_rem1")
        lz_rem2 = consts.tile([128, N_DC, C], F32, name="lz_rem2")
        lz_splits = [lz_hi, lz_mid, lz_lo]

        def half_ep(h):
            """ep[:, half] = exp(M.T @ yT_half + ebias)."""
            jsl = slice(h * JC, (h + 1) * JC)
            ps_u = sm_tile(K, JC)
            for dc in range(N_DC):
                nc.tensor.matmul(ps_u, m_sb[:, dc, :], yT[dc][:, jsl],
                                 start=(dc == 0), stop=(dc == N_DC - 1))
            nc.scalar.activation(ep[:, jsl], ps_u,
                                 mybir.ActivationFunctionType.Exp,
                                 bias=ebias, scale=1.0)

        def chunk_lz(lc):
            """lz chunk lc + bf16 splits + flatten into rhs rows 96..98."""
            h, lc4 = lc // 4, lc % 4
            csl = slice(lc * 128, (lc + 1) * 128)
            ps_nd = sm_tile(128, C + 1)
            nc.tensor.matmul(ps_nd, ep[:, csl], gb)
            recip = small.tile([128, 1], F32, name="recip", tag="recip")
            nc.vector.reciprocal(recip, ps_nd[:, C : C + 1])
            nc.vector.scalar_tensor_tensor(
                lz_sb[:, lc, :], ps_nd[:, 0:C], recip, csb_rep,
                op0=mybir.AluOpType.mult, op1=mybir.AluOpType.add)
            nc.vector.tensor_copy(lz_hi[:, lc, :], lz_sb[:, lc, :])
            nc.vector.tensor_sub(lz_rem1[:, lc, :], lz_sb[:, lc, :],
                                 lz_hi[:, lc, :])
            nc.vector.tensor_copy(lz_mid[:, lc, :], lz_rem1[:, lc, :])
            nc.vector.tensor_sub(lz_rem2[:, lc, :], lz_rem1[:, lc, :],
                                 lz_mid[:, lc, :])
            nc.vector.tensor_copy(lz_lo[:, lc, :], lz_rem2[:, lc, :])
            for s in range(3):
                nc.sync.dma_start(
                    out=rhs[h][96 + s : 97 + s, lc4 * 128 * C : (lc4 + 1) * 128 * C],
                    in_=lz_splits[s][:, lc, :],
                )

        def chunk_ly(ic):
            """lyT cols ic*128.. -> bf16 splits into ly_lhsT."""
            csl = slice(ic * 128, (ic + 1) * 128)
            ps_lyc = sm_tile(C, 128)
            for dc in range(N_DC):
                nc.tensor.matmul(ps_lyc, csy_sb[:, dc, :], yT[dc][:, csl],
                                 start=(dc == 0), stop=(dc == N_DC - 1))
            hi_b = small.tile([C, 128], BF16, name="hi_b", tag="hi_b")
            mid_b = small.tile([C, 128], BF16, name="mid_b", tag="mid_b")
            lo_b = small.tile([C, 128], BF16, name="lo_b", tag="lo_b")
            rem1 = small.tile([C, 128], F32, name="rem1", tag="rem1")
            rem2 = small.tile([C, 128], F32, name="rem2", tag="rem2")
            nc.scalar.copy(hi_b, ps_lyc)
            nc.vector.tensor_sub(rem1, ps_lyc, hi_b)
            nc.scalar.copy(mid_b, rem1)
            nc.vector.tensor_sub(rem2, rem1, mid_b)
            nc.scalar.copy(lo_b, rem2)
            nc.sync.dma_start(out=ly_lhsT[0:C, csl], in_=hi_b)
            nc.sync.dma_start(out=ly_lhsT[32 : 32 + C, csl], in_=mid_b)
            nc.sync.dma_start(out=ly_lhsT[64 : 64 + C, csl], in_=lo_b)

        def outer_sum(h, ic):
            lhs = ly_lhsT[:, ic * 128 : (ic + 1) * 128]
            for qg in range(Q_N // OUT_Q):
                ob = outpool.tile([128, OUT_W], F32, name="ob", tag="ob")
                for qq in range(OUT_Q):
                    q = qg * OUT_Q + qq
                    po = out_ps.tile([128, 512], F32, name="po", tag="po")
                    nc.tensor.matmul(po, lhs, rhs[h][:, q * 512 : (q + 1) * 512])
                    dst = ob[:, qq * 512 : (qq + 1) * 512]
                    if q % 2 == 0:
                        nc.vector.tensor_copy(dst, po)
                    else:
                        nc.scalar.copy(dst, po)
                nc.sync.dma_start(
                    out=out[ic * 128 : (ic + 1) * 128,
                            h * RHS_W + qg * OUT_W : h * RHS_W + (qg + 1) * OUT_W],
                    in_=ob,
                )

        # h=0 attention + ly, then phase-2 h=0 interleaved with the h=1
        # pipeline (so h=1's small ops don't queue behind a full half's
        # PSUM->SBUF copies on the in-order engine queues), then phase-2 h=1.
        half_ep(0)
        for lc in range(4):
            chunk_lz(lc)
        for ic in range(4):
            chunk_ly(ic)
        # h=1 y loads, throttled to here so they don't steal early DMA
        # bandwidth from the critical h=0 path
        for dc in range(N_DC):
            nc.scalar.dma_start(
                out=yT[dc][:, JC:L],
                in_=yt_d[dc * 128 : (dc + 1) * 128, JC:L],
            )
        outer_sum(0, 0)
        half_ep(1)
        outer_sum(0, 1)
        for lc in (4, 5):
            chunk_lz(lc)
        outer_sum(0, 2)
        for lc in (6, 7):
            chunk_lz(lc)
        outer_sum(0, 3)
        for ic in range(N_IC):
            outer_sum(1, ic)


_NC_CACHE = None


def _get_nc():
    global _NC_CACHE
    if _NC_CACHE is None:
        _NC_CACHE = _build_program()
    return _NC_CACHE


def _host_weights(inputs):
    """Fold the weight-only algebra on the host (float64 for headroom)."""
    dic = np.asarray(inputs["dic"], np.float64)
    prior = np.asarray(inputs["prior"], np.float64)
    wy_w = np.asarray(inputs["Wy_w"], np.float64)
    wy_b = np.asarray(inputs["Wy_b"], np.float64)
    wz_w = np.asarray(inputs["Wz_w"], np.float64)
    wz_b = np.asarray(inputs["Wz_b"], np.float64)
    cs_w = np.asarray(inputs["cs_w"], np.float64)
    cs_b = np.asarray(inputs["cs_b"], np.float32)

    zdts = (wz_w @ dic.T + wz_b[:, None]) * float(SCALE)   # [C, K]
    m = (wy_w.T @ zdts).astype(np.float32)                 # [D, K]
    m = np.ascontiguousarray(m.reshape(N_DC, 128, K).transpose(1, 0, 2))
    ebias = (wy_b @ zdts).astype(np.float32)               # [K]
    g = (prior[:, None] * dic) @ cs_w[:, D:].T             # [K, C]
    gb = np.concatenate([g, np.ones((K, 1))], axis=1).astype(np.float32)
    csyT = cs_w[:, :D].T.astype(np.float32)                # [D, C]
    csyT = np.ascontiguousarray(csyT.reshape(N_DC, 128, C).transpose(1, 0, 2))
    import ml_dtypes
    icorner = np.zeros((32, RHS_W), ml_dtypes.bfloat16)
    for c in range(C):
        icorner[c, c::C] = 1.0
    return {
        "icorner": icorner,
        "M": m,
        "csyT": csyT,
        "gb": np.ascontiguousarray(gb),
        "ebias": np.ascontiguousarray(ebias),
        "cs_b": np.ascontiguousarray(cs_b),
    }


def make_in_maps(inputs):
    x = np.asarray(inputs["x"], dtype=np.float32)
    w = _host_weights(inputs)
    xT = [np.ascontiguousarray(x[b].T) for b in range(B)]  # [D, L] each
    in_maps = []
    for core in range(8):
        b, ihalf = core % B, core // B
        if ihalf == 0:
            yt = xT[b]
        else:
            yt = np.ascontiguousarray(
                np.concatenate([xT[b][:, HALF_L:], xT[b][:, :HALF_L]], axis=1)
            )
        in_maps.append({"yT": yt, **w})
    return in_maps


def assemble(results):
    out = np.empty((B, L, L, C), dtype=np.float32)
    for core in range(8):
        b, ihalf = core % B, core // B
        # device output: [512 i_local, 2 processed-half, 512 j_local, C];
        # processed half 0 covers real j-half `ihalf`, half 1 the other.
        r = results[core]["out_loc"].reshape(HALF_L, 2, JC, C)
        dst = out[b, ihalf * HALF_L : (ihalf + 1) * HALF_L]
        dst[:, ihalf * JC : (ihalf + 1) * JC] = r[:, 0]
        dst[:, (1 - ihalf) * JC : (2 - ihalf) * JC] = r[:, 1]
    return out.reshape(B, L * L, C)


def _install_trace_support():
    """The agent image's antenv lacks axon_hooks, so boot() skipped NTFF hook
    install. Recreate the module and register the ctypes-based hook; also stub
    the S3 artifact upload (no creds in this container)."""
    import types

    if sys.modules.get("antenv.axon_hooks") is None:
        mod = types.ModuleType("antenv.axon_hooks")
        _hook = [None]
        mod.set_axon_ntff_profile_hook = lambda h: _hook.__setitem__(0, h)
        mod.get_axon_ntff_profile_hook = lambda: _hook[0]
        sys.modules["antenv.axon_hooks"] = mod
        import antenv

        antenv.axon_hooks = mod
    import antenv.axon_hooks as ah

    if ah.get_axon_ntff_profile_hook() is None:
        from trn_agent_boot.trn_boot import _ntff_profile_via_ctypes

        ah.set_axon_ntff_profile_hook(
            _ntff_profile_via_ctypes("/opt/axon/libaxon_pjrt.so")
        )
    import concourse.bass_utils as bu

    bu.upload_artifacts = lambda tmpdir: tmpdir


def run(inputs, trace=False, **kw):
    from concourse.bass_utils import run_bass_kernel_spmd

    if trace:
        _install_trace_support()
    nc = _get_nc()
    res = run_bass_kernel_spmd(
        nc, make_in_maps(inputs), core_ids=list(range(8)), trace=trace, **kw
    )
    return assemble(res.results), res


def kernel(**inputs) -> np.ndarray:
    out, _ = run(inputs, trace=False)
    return out
